# revision 2
# baseline (speedup 1.0000x reference)
"""Trainium2 Bass kernel for nn_MAE_CalcLoss_Raw (masked MSE loss).

reference math:
    masked   = mean_b[ mean_{i,d} (outputs[b, mask_id[b,i], d]   - orig[b, mask_id[b,i], d])^2 ]
    unmasked = mean_b[ mean_{i,d} (outputs[b, unmask_id[b,i], d] - orig[b, unmask_id[b,i], d])^2 ]
    loss = masked + 0.1 * unmasked

Rewrite: gathering rows by index (with repeats) is a weighted sum over
all (b, s) rows.  With cnt_m[b,s] = #occurrences of s in mask_id[b],
cnt_u likewise:

    loss = sum_{b,s} w[b,s] * ||outputs[b,s,:] - orig[b,s,:]||^2
    w[b,s] = cnt_m[b,s]/(B*Nm*D) + ALPHA*cnt_u[b,s]/(B*Nu*D)

The device kernel streams both [B,S,D] tensors (memory-bound; 512 MB
total), computes per-row sum-of-squared-diff on DVE (subtract) + ACT
(square + per-row accumulate), applies the tiny host-computed weight
matrix, and returns one [128] partial vector per core.  Host sums 8*128
floats.  Data-parallel over B: 8 samples per core on 8 cores.
"""

import numpy as np

ALPHA = 0.1
B, S, D = 64, 2048, 512
NM, NU = 1536, 512
N_CORES = 8
BPC = B // N_CORES            # samples per core
R = BPC * S                   # rows per core = 16384
GROUPS = 8                    # 128-row groups per tile
TILE_ROWS = GROUPS * 128      # 1024 rows per tile (2 MB per tensor)
N_TILES = R // TILE_ROWS      # 16

_CACHE: dict = {}


def _build_nc():
    import concourse.bacc as bacc
    import concourse.bass as bass
    import concourse.tile as tile
    import concourse.mybir as mybir

    f32 = mybir.dt.float32
    nc = bacc.Bacc(
        "TRN2",
        target_bir_lowering=False,
        debug=False,
        enable_asserts=False,
        num_devices=N_CORES,
    )
    x_d = nc.dram_tensor("x", [R, D], f32, kind="ExternalInput").ap()
    y_d = nc.dram_tensor("y", [R, D], f32, kind="ExternalInput").ap()
    w_d = nc.dram_tensor("w", [128, N_TILES * GROUPS], f32, kind="ExternalInput").ap()
    p_d = nc.dram_tensor("partial", [128, 1], f32, kind="ExternalOutput").ap()

    with tile.TileContext(nc) as tc:
        with (
            tc.tile_pool(name="io", bufs=3) as io,
            tc.tile_pool(name="acc", bufs=1) as acc,
        ):
            w_sb = acc.tile([128, N_TILES * GROUPS], f32, tag="w")
            nc.sync.dma_start(w_sb[:], w_d[:])
            racc = acc.tile([128, N_TILES * GROUPS], f32, tag="racc")

            for i in range(N_TILES):
                xt = io.tile([128, GROUPS, D], f32, tag="x")
                nc.sync.dma_start(
                    xt[:],
                    x_d[bass.ts(i, TILE_ROWS), :].rearrange(
                        "(g p) d -> p g d", g=GROUPS, p=128
                    ),
                )
                yt = io.tile([128, GROUPS, D], f32, tag="y")
                nc.sync.dma_start(
                    yt[:],
                    y_d[bass.ts(i, TILE_ROWS), :].rearrange(
                        "(g p) d -> p g d", g=GROUPS, p=128
                    ),
                )
                # diff in place on DVE
                nc.vector.tensor_sub(xt[:], xt[:], yt[:])
                # square + per-row (per 512-elem group) accumulate on ACT
                for g in range(GROUPS):
                    j = i * GROUPS + g
                    nc.scalar.activation(
                        xt[:, g, :],
                        xt[:, g, :],
                        mybir.ActivationFunctionType.Square,
                        accum_out=racc[:, j : j + 1],
                    )

            prod = acc.tile([128, N_TILES * GROUPS], f32, tag="prod")
            nc.vector.tensor_mul(prod[:], racc[:], w_sb[:])
            part = acc.tile([128, 1], f32, tag="part")
            nc.vector.tensor_reduce(
                part[:], prod[:], axis=mybir.AxisListType.X, op=mybir.AluOpType.add
            )
            nc.sync.dma_start(p_d[:], part[:])

    nc.compile()
    return nc


def _get_nc():
    if "nc" not in _CACHE:
        _CACHE["nc"] = _build_nc()
    return _CACHE["nc"]


def _weights(mask_id: np.ndarray, unmask_id: np.ndarray) -> np.ndarray:
    """w[b,s] from index histograms, float64 [B,S]."""
    rows = np.arange(B)[:, None]
    cm = np.zeros((B, S), np.float64)
    np.add.at(cm, (rows, mask_id.astype(np.int64)), 1.0)
    cu = np.zeros((B, S), np.float64)
    np.add.at(cu, (rows, unmask_id.astype(np.int64)), 1.0)
    return cm / (B * NM * D) + ALPHA * cu / (B * NU * D)


def _in_maps(outputs, orig_image, mask_id, unmask_id):
    w = _weights(mask_id, unmask_id)  # [B,S] f64
    x = np.ascontiguousarray(np.asarray(outputs, dtype=np.float32)).reshape(B * S, D)
    y = np.ascontiguousarray(np.asarray(orig_image, dtype=np.float32)).reshape(B * S, D)
    maps = []
    for c in range(N_CORES):
        w_c = w[c * BPC : (c + 1) * BPC].reshape(R)
        # racc[p, i*G+g] holds row r = i*TILE_ROWS + g*128 + p
        W_c = (
            w_c.reshape(N_TILES, GROUPS, 128)
            .transpose(2, 0, 1)
            .reshape(128, N_TILES * GROUPS)
            .astype(np.float32)
        )
        maps.append(
            {
                "x": x[c * R : (c + 1) * R],
                "y": y[c * R : (c + 1) * R],
                "w": np.ascontiguousarray(W_c),
            }
        )
    return maps


def _run(inputs: dict, trace: bool = False, **kw):
    from concourse.bass_utils import run_bass_kernel_spmd

    nc = _get_nc()
    maps = _in_maps(**inputs)
    res = run_bass_kernel_spmd(
        nc, maps, list(range(N_CORES)), trace=trace, **kw
    )
    total = np.float64(0.0)
    for c in range(N_CORES):
        total += np.asarray(res.results[c]["partial"], dtype=np.float64).sum()
    return np.asarray(total, dtype=np.float32), res


def kernel(outputs, orig_image, mask_id, unmask_id):
    out, _ = _run(
        {
            "outputs": outputs,
            "orig_image": orig_image,
            "mask_id": mask_id,
            "unmask_id": unmask_id,
        }
    )
    return out
